# revision 7
# baseline (speedup 1.0000x reference)
"""CoordinateLoss (masked Kabsch + Huber) on 8 Trainium2 NeuronCores.

Sharding: data-parallel over batch. B=256 samples -> 32 per core.
Two SPMD launches with a tiny host step (256x 3x3 SVDs) between them:

  Pass 1 (device): per-sample masked covariance M = sum(mask*p_i*q_j),
     sums Sp/St via a single fp32r matmul per 128-point chunk:
     lhsT = [mp | ones] (128x97), rhs = [mt | mp] (128x192), accumulated
     across all 16384 points into one PSUM tile [97,192].
  Host: H = M - Sp St^T / cnt, batched SVD -> R,t (reference formula).
  Pass 2 (device): aligned = R @ (mask*pred) via block-diagonal R matmul
     (psum [points,(b,i)]), d = aligned - mask*(target - t), then
     huber sum = 0.5*sum(d^2) - 0.5*sum(relu(|d|-1)^2)  (exact for delta=1).

All mask multiplies are folded on the host (mask^2 == mask), so the device
does no masking work; cnt comes from a host sum.
"""

import numpy as np

import concourse.bacc as bacc
import concourse.mybir as mybir
from concourse.tile import TileContext
from concourse.bass_utils import run_bass_kernel_spmd

B = 256
S = 16384
NCORES = 8
BPC = B // NCORES          # samples per core = 32
KCOLS = 3 * BPC            # 96  (b, j) columns
PTS_PER_CHUNK = 128
CHUNKS = S // PTS_PER_CHUNK            # 128
SC = 4                                  # chunks per super-chunk
NSC = CHUNKS // SC                      # 32 super-chunks
F32 = mybir.dt.float32
F32R = mybir.dt.float32r

_cache = {}


def _build_pass1():
    nc = bacc.Bacc("TRN2", target_bir_lowering=False, debug=False)
    # [mt | mp | ones] layout: col 3b+j inside each 96-block, last col = 1.0
    mpt = nc.dram_tensor("mpt", [S, 2 * KCOLS + 1], F32R, kind="ExternalInput")
    stats = nc.dram_tensor("stats", [KCOLS + 1, 2 * KCOLS], F32, kind="ExternalOutput")

    mpt_v = mpt[:].rearrange("(n c p) k -> n p c k", p=PTS_PER_CHUNK, c=SC)

    with TileContext(nc) as tc:
        with (
            tc.tile_pool(name="io", bufs=3) as io,
            tc.tile_pool(name="fin", bufs=1) as fin,
            tc.tile_pool(name="psum", bufs=1, space="PSUM") as psum,
        ):
            acc = psum.tile([KCOLS + 1, 2 * KCOLS], F32)
            for sc in range(NSC):
                t = io.tile([PTS_PER_CHUNK, SC, 2 * KCOLS + 1], F32R, tag="mpt_t")
                nc.sync.dma_start(t[:], mpt_v[sc])
                for c in range(SC):
                    lhsT = t[:, c, KCOLS : 2 * KCOLS + 1]   # [128, 97] = [mp | ones]
                    rhs = t[:, c, 0 : 2 * KCOLS]            # [128, 192] = [mt | mp]
                    nc.tensor.matmul(
                        acc[:],
                        lhsT,
                        rhs,
                        start=(sc == 0 and c == 0),
                        stop=(sc == NSC - 1 and c == SC - 1),
                    )
            out_t = fin.tile([KCOLS + 1, 2 * KCOLS], F32)
            nc.vector.tensor_copy(out_t[:], acc[:])
            nc.sync.dma_start(stats[:], out_t[:])
    nc.compile()
    return nc


def _build_pass2():
    nc = bacc.Bacc("TRN2", target_bir_lowering=False, debug=False)
    p2 = nc.dram_tensor("p2", [KCOLS, S], F32R, kind="ExternalInput")     # mask*pred, (b,j) rows
    q2 = nc.dram_tensor("q2", [S, KCOLS], F32, kind="ExternalInput")     # mask*(target - t)
    rbd = nc.dram_tensor("rbd", [KCOLS, KCOLS], F32R, kind="ExternalInput")
    out = nc.dram_tensor("out", [128, 2], F32, kind="ExternalOutput")

    q2_v = q2[:].rearrange("(n c p) k -> n p c k", p=PTS_PER_CHUNK, c=SC)
    W = SC * KCOLS  # 384

    with TileContext(nc) as tc:
        with (
            tc.tile_pool(name="const", bufs=1) as const,
            tc.tile_pool(name="io", bufs=3) as io,
            tc.tile_pool(name="work", bufs=3) as work,
            tc.tile_pool(name="accp", bufs=1) as accp,
            tc.tile_pool(name="psum", bufs=4, space="PSUM") as psum,
        ):
            rbd_t = const.tile([KCOLS, KCOLS], F32R)
            nc.sync.dma_start(rbd_t[:], rbd[:])
            acc1 = accp.tile([128, NSC], F32)
            acc2 = accp.tile([128, NSC], F32)

            for sc in range(NSC):
                p2t = io.tile([KCOLS, SC * PTS_PER_CHUNK], F32R, tag="p2t")
                nc.sync.dma_start(
                    p2t[:], p2[:, sc * SC * PTS_PER_CHUNK : (sc + 1) * SC * PTS_PER_CHUNK]
                )
                q2t = io.tile([PTS_PER_CHUNK, SC, KCOLS], F32, tag="q2t")
                nc.sync.dma_start(q2t[:], q2_v[sc])

                pa = psum.tile([PTS_PER_CHUNK, W], F32, tag="pa")
                for c in range(SC):
                    nc.tensor.matmul(
                        pa[:, c * KCOLS : (c + 1) * KCOLS],
                        p2t[:, c * PTS_PER_CHUNK : (c + 1) * PTS_PER_CHUNK],
                        rbd_t[:],
                        start=True,
                        stop=True,
                    )
                # huber(d) = c*d - 0.5*c^2 with c = clamp(d, -1, 1)  (delta=1)
                d = work.tile([128, W], F32, tag="d")
                q2f = q2t[:].rearrange("p c k -> p (c k)")
                nc.vector.tensor_tensor(d[:], pa[:], q2f, mybir.AluOpType.subtract)
                c_t = work.tile([128, W], F32, tag="c_t")
                nc.gpsimd.tensor_scalar(
                    c_t[:], d[:], 1.0, -1.0,
                    mybir.AluOpType.min, mybir.AluOpType.max,
                )
                j1 = work.tile([128, W], F32, tag="j1")
                nc.vector.scalar_tensor_tensor(
                    out=j1[:], in0=c_t[:], scalar=1.0, in1=d[:],
                    op0=mybir.AluOpType.mult, op1=mybir.AluOpType.mult,
                    accum_out=acc1[:, sc : sc + 1],
                )
                j2 = work.tile([128, W], F32, tag="j2")
                nc.scalar.activation(
                    j2[:], c_t[:], mybir.ActivationFunctionType.Square,
                    accum_out=acc2[:, sc : sc + 1],
                )

            fin = accp.tile([128, 2], F32)
            nc.vector.tensor_reduce(
                fin[:, 0:1], acc1[:], axis=mybir.AxisListType.X, op=mybir.AluOpType.add
            )
            nc.vector.tensor_reduce(
                fin[:, 1:2], acc2[:], axis=mybir.AxisListType.X, op=mybir.AluOpType.add
            )
            nc.sync.dma_start(out[:], fin[:])
    nc.compile()
    return nc


def _get_ncs():
    if "nc1" not in _cache:
        _cache["nc1"] = _build_pass1()
        _cache["nc2"] = _build_pass2()
    return _cache["nc1"], _cache["nc2"]


def kernel(pred_coords, target_coords, mask):
    nc1, nc2 = _get_ncs()
    pred = np.ascontiguousarray(pred_coords, dtype=np.float32)
    targ = np.ascontiguousarray(target_coords, dtype=np.float32)
    maskf = mask.astype(np.float32)

    mp = pred * maskf[..., None]          # [B, S, 3]
    mt = targ * maskf[..., None]
    cnt = maskf.sum(axis=1)               # [B]

    # ---- pass 1: per-sample M, Sp, St ----
    in1 = []
    for c in range(NCORES):
        sl = slice(c * BPC, (c + 1) * BPC)
        mp1 = mp[sl].transpose(1, 0, 2).reshape(S, KCOLS)   # (s, 3b+j)
        mt1 = mt[sl].transpose(1, 0, 2).reshape(S, KCOLS)
        ones = np.ones((S, 1), np.float32)
        mpt = np.ascontiguousarray(np.concatenate([mt1, mp1, ones], axis=1))
        in1.append({"mpt": mpt})
    res1 = run_bass_kernel_spmd(nc1, in1, core_ids=list(range(NCORES)))

    idx = np.arange(BPC)
    M = np.empty((B, 3, 3), np.float64)
    Sp = np.empty((B, 3), np.float64)
    St = np.empty((B, 3), np.float64)
    for c in range(NCORES):
        st = res1.results[c]["stats"]
        sl = slice(c * BPC, (c + 1) * BPC)
        M[sl] = st[:KCOLS, :KCOLS].reshape(BPC, 3, BPC, 3)[idx, :, idx, :]
        St[sl] = st[KCOLS, :KCOLS].reshape(BPC, 3)
        Sp[sl] = st[KCOLS, KCOLS:].reshape(BPC, 3)

    # ---- host: Kabsch from the reductions (reference formula, f64) ----
    cnt64 = cnt.astype(np.float64)
    cp = Sp / cnt64[:, None]
    ct = St / cnt64[:, None]
    H = M - Sp[:, :, None] * St[:, None, :] / cnt64[:, None, None]
    U, _, Vt = np.linalg.svd(H)
    R = np.einsum("bji,bkj->bik", Vt, U)
    sign = np.where(np.linalg.det(R) < 0, -1.0, 1.0)
    Vt[:, -1, :] *= sign[:, None]
    R = np.einsum("bji,bkj->bik", Vt, U)
    t = ct - np.einsum("bij,bj->bi", R, cp)

    R32 = R.astype(np.float32)
    t32 = t.astype(np.float32)

    # ---- pass 2: masked huber of (R p + t - q) ----
    in2 = []
    for c in range(NCORES):
        sl = slice(c * BPC, (c + 1) * BPC)
        p2 = np.ascontiguousarray(mp[sl].transpose(0, 2, 1).reshape(KCOLS, S))
        q2 = np.ascontiguousarray(
            (mt[sl] - maskf[sl][..., None] * t32[sl][:, None, :])
            .transpose(1, 0, 2).reshape(S, KCOLS)
        )
        rbd = np.zeros((BPC, 3, BPC, 3), np.float32)
        rbd[idx, :, idx, :] = R32[sl].transpose(0, 2, 1)  # rbd[(b,j),(b,i)] = R[i,j]
        in2.append({"p2": p2, "q2": q2, "rbd": rbd.reshape(KCOLS, KCOLS)})
    res2 = run_bass_kernel_spmd(nc2, in2, core_ids=list(range(NCORES)))

    s1 = 0.0
    s2 = 0.0
    for c in range(NCORES):
        o = res2.results[c]["out"].astype(np.float64)
        s1 += o[:, 0].sum()   # sum(c*d)
        s2 += o[:, 1].sum()   # sum(c^2)
    loss = (s1 - 0.5 * s2) / cnt64.sum()
    return np.array(loss, dtype=np.float32)
